# revision 2
# baseline (speedup 1.0000x reference)
"""AttentiveMatchingLayer TRN2 kernel.

Math (per batch, validated against the jax reference):
  ssa[t] = sum_d a[t,d]^2 ; ssb likewise ; stok = 1/sqrt(ssa*ssb)
  as = a * stok[:,None]                     # carries BOTH l2 norms
  alpha[d,e] = sum_t b[t,d] * as[t,e]       # == ref alpha (norms folded)
  s_al[e] = 1/sqrt(sum_d alpha[d,e]^2)
  hmT[e,t] = sum_d alpha[d,e] * b[t,d]      # s_al folded into w2 scalings
     (differs from ref hmean by a per-token positive factor 1/rb[t],
      which cancels in the final cosine)
  num[t,p] = sum_d (a*hmT) (W2*s_al) ; sa = sum_d a^2 W2 ; sh = sum_d hmT^2 (W2*s_al^2)
  persp = num / sqrt((sa+eps)*(sh+eps))
Sharding: data-parallel over batch B=32 across 8 cores (4 batches/core).

Implementation notes:
- float16 streaming tensors (10-bit mantissa: ~3e-4 end-to-end error vs the
  f32 reference); f32 PSUM accumulation; f32 norms/final math.
- Phase-major over the core's 4 batches: each engine phase is one dense 4x
  burst (keeps the PE HAM-warm and hides the serial norm chains).
- Per-token / per-column sumsq land token-major on partitions by riding an
  extra ones-column of w2t (ssa), by square+reduce in [t,d] layout (ssb),
  or as rhs=ones matmuls (s_al) — no partition scatters anywhere.
"""

import numpy as np
from contextlib import ExitStack

import concourse.bacc as bacc
import concourse.bass as bass
import concourse.tile as tile
from concourse import masks, mybir

B, T, D, P = 32, 1024, 256, 20
PA = P + 1         # w2t augmented with a ones column (-> ssa)
N_CORES = 8
NB = B // N_CORES  # batches per core
TC = T // 128      # 8 token chunks
DC = D // 128      # 2 d chunks
F32 = mybir.dt.float32
F16 = mybir.dt.float16
EPS = 1e-12
Square = mybir.ActivationFunctionType.Square
Sqrt = mybir.ActivationFunctionType.Sqrt


def build_kernel():
    nc = bacc.Bacc("TRN2", target_bir_lowering=False, debug=False,
                   num_devices=N_CORES)
    a_in = nc.declare_dram_parameter("a", [NB, T, D], F32, isOutput=False)
    b_in = nc.declare_dram_parameter("b", [NB, T, D], F32, isOutput=False)
    w2t_in = nc.declare_dram_parameter("w2t", [D, PA], F32, isOutput=False)
    out_d = nc.declare_dram_parameter("out", [NB, T, P], F32, isOutput=True)

    with tile.TileContext(nc) as tc, ExitStack() as ctx:
        consts = ctx.enter_context(tc.tile_pool(name="consts", bufs=1))
        p4 = ctx.enter_context(tc.tile_pool(name="p4", bufs=NB))
        pscr = ctx.enter_context(tc.tile_pool(name="pscr", bufs=2))
        ps = ctx.enter_context(tc.tile_pool(name="ps", bufs=8, space="PSUM"))

        identf = consts.tile([128, 128], F32)
        masks.make_identity(nc, identf[:])
        ident = consts.tile([128, 128], F16)
        nc.vector.tensor_copy(ident[:], identf[:])
        ones = consts.tile([128, 1], F16)
        nc.vector.memset(ones[:], 1.0)
        eps_sb = consts.tile([128, 1], F32)
        nc.vector.memset(eps_sb[:], EPS)
        w2t = consts.tile([128, DC, PA], F16)
        nc.gpsimd.dma_start(
            out=w2t[:], in_=w2t_in.ap().rearrange("(dc p) w -> p dc w", p=128))

        NBR = range(NB)
        # ---- loads (f32 -> f16 cast in SWDGE DMA) ----
        a_sb, b_sb = [], []
        for b in NBR:
            a_sb.append(p4.tile([128, TC, D], F16, tag="a_sb", name=f"a_sb{b}"))
            nc.gpsimd.dma_start(
                out=a_sb[b][:],
                in_=a_in.ap()[b].rearrange("(p c) d -> p c d", p=128))
            b_sb.append(p4.tile([128, TC, D], F16, tag="b_sb", name=f"b_sb{b}"))
            nc.gpsimd.dma_start(
                out=b_sb[b][:],
                in_=b_in.ap()[b].rearrange("(p c) d -> p c d", p=128))

        # ---- ssb: square (GpSimd) + reduce over d (DVE), [t, d] layout ----
        ssb_sb, stok, sa_sb = [], [], []
        for b in NBR:
            sq = pscr.tile([128, TC, D], F16, tag="sq_scr", name=f"sq{b}")
            nc.gpsimd.tensor_mul(
                sq[:].rearrange("p c d -> p (c d)"),
                b_sb[b][:].rearrange("p c d -> p (c d)"),
                b_sb[b][:].rearrange("p c d -> p (c d)"))
            ssb_sb.append(p4.tile([128, TC], F32, tag="ssb", name=f"ssb{b}"))
            nc.vector.reduce_sum(ssb_sb[b][:], sq[:], axis=mybir.AxisListType.X)

        # ---- transposes (PE) + prompt evacuation ----
        aT_sb, bT_sb, asq_sb, as_sb = [], [], [], []
        for b in NBR:
            aT_ps = [ps.tile([128, 1024], F16, tag="ps", name=f"aT_ps{b}_{i}")
                     for i in range(DC)]
            bT_ps = [ps.tile([128, 1024], F16, tag="ps", name=f"bT_ps{b}_{i}")
                     for i in range(DC)]
            for dc in range(DC):
                for c in range(TC):
                    nc.tensor.transpose(
                        out=aT_ps[dc][:, c * 128:(c + 1) * 128],
                        in_=a_sb[b][:, c, dc * 128:(dc + 1) * 128],
                        identity=ident[:])
                    nc.tensor.transpose(
                        out=bT_ps[dc][:, c * 128:(c + 1) * 128],
                        in_=b_sb[b][:, c, dc * 128:(dc + 1) * 128],
                        identity=ident[:])
            aT_sb.append(p4.tile([128, DC, T], F16, tag="aT_sb", name=f"aT{b}"))
            bT_sb.append(p4.tile([128, DC, T], F16, tag="bT_sb", name=f"bT{b}"))
            asq_sb.append(p4.tile([128, DC, T], F16, tag="asq_sb", name=f"asq{b}"))
            for dc in range(DC):
                nc.scalar.copy(aT_sb[b][:, dc, :], aT_ps[dc][:])
                nc.scalar.copy(bT_sb[b][:, dc, :], bT_ps[dc][:])
                nc.scalar.activation(asq_sb[b][:, dc, :], aT_ps[dc][:], Square)

            # early sa matmul right behind this batch's transposes: keeps PE
            # dense while the stok chain of earlier batches runs on DVE/ACT
            sa_ps = [ps.tile([128, (TC // 2) * PA], F32, tag="ps",
                             name=f"sa_ps{b}_{i}") for i in range(2)]
            for c in range(TC):
                for dc in range(DC):
                    nc.tensor.matmul(
                        sa_ps[c % 2][:, (c // 2) * PA:(c // 2) * PA + PA],
                        lhsT=asq_sb[b][:, dc, c * 128:(c + 1) * 128],
                        rhs=w2t[:, dc, :],
                        start=(dc == 0), stop=(dc == DC - 1))
            sa_sb.append(p4.tile([128, 2, (TC // 2) * PA], F32, tag="sa_sb",
                                 name=f"sa_sb{b}"))
            for h in range(2):
                nc.vector.tensor_copy(sa_sb[b][:, h, :], sa_ps[h][:])

            # stok = 1/sqrt(ssa*ssb) ; as = a*stok (chains overlap next batch)
            st = p4.tile([128, TC], F32, tag="stok", name=f"stok{b}")
            stok.append(st)
            for h in range(2):
                nc.vector.tensor_mul(
                    st[:, h::2],
                    sa_sb[b][:, h, :].rearrange("q (c w) -> q c w", w=PA)[:, :, P],
                    ssb_sb[b][:, h::2])
            nc.scalar.activation(st[:], st[:], Sqrt)
            nc.vector.reciprocal(st[:], st[:])
            as_sb.append(p4.tile([128, TC, D], F16, tag="as_sb", name=f"as_sb{b}"))
            for c in range(TC):
                nc.vector.tensor_scalar_mul(
                    as_sb[b][:, c, :], a_sb[b][:, c, :], st[:, c:c + 1])

        # ---- alpha[d, e] = sum_t b[t,d] as[t,e] (PE, bank ping-pong) ----
        alpha_sb, alsq_sb = [], []
        for b in NBR:
            alpha_ps = [ps.tile([128, 256], F32, tag="ps", name=f"al_ps{b}_{i}")
                        for i in range(DC)]
            for c in range(TC):
                for dc in range(DC):
                    nc.tensor.matmul(
                        alpha_ps[dc][:],
                        lhsT=b_sb[b][:, c, dc * 128:(dc + 1) * 128],
                        rhs=as_sb[b][:, c, :],
                        start=(c == 0), stop=(c == TC - 1))
            alpha_sb.append(p4.tile([128, DC, 256], F16, tag="alpha_sb",
                                    name=f"alpha_sb{b}"))
            alsq_sb.append(p4.tile([128, DC, 256], F16, tag="alsq_sb",
                                   name=f"alsq_sb{b}"))
            for dc in range(DC):
                nc.vector.tensor_copy(alpha_sb[b][:, dc, :], alpha_ps[dc][:])
                nc.scalar.activation(alsq_sb[b][:, dc, :], alpha_ps[dc][:], Square)

        # ---- s_al + folded w2 scalings ----
        w2sal, w2sal2 = [], []
        for b in NBR:
            sal_ps = ps.tile([128, 2], F32, tag="ps", name=f"sal_ps{b}")
            for ec in range(2):
                for dc in range(DC):
                    nc.tensor.matmul(
                        sal_ps[:, ec:ec + 1],
                        lhsT=alsq_sb[b][:, dc, ec * 128:(ec + 1) * 128],
                        rhs=ones[:],
                        start=(dc == 0), stop=(dc == DC - 1))
            sal = p4.tile([128, 2], F32, tag="sal", name=f"sal{b}")
            nc.vector.tensor_copy(sal[:], sal_ps[:])
            nc.scalar.activation(sal[:], sal[:], Sqrt)
            nc.vector.reciprocal(sal[:], sal[:])
            ws = p4.tile([128, DC, P], F16, tag="w2sal", name=f"w2sal{b}")
            ws2 = p4.tile([128, DC, P], F16, tag="w2sal2", name=f"w2sal2{b}")
            for dc in range(DC):
                nc.vector.tensor_scalar_mul(
                    ws[:, dc, :], w2t[:, dc, 0:P], sal[:, dc:dc + 1])
                nc.vector.tensor_scalar_mul(
                    ws2[:, dc, :], ws[:, dc, :], sal[:, dc:dc + 1])
            w2sal.append(ws)
            w2sal2.append(ws2)

        # ---- hmT (PE) ; prod = aT*hmT (DVE) ; hmsq = hmT^2 (ACT) ----
        prod_sb, hmsq_sb = [], []
        for b in NBR:
            prod_sb.append(p4.tile([128, 2, T], F16, tag="prod_sb",
                                   name=f"prod{b}"))
            hmsq_sb.append(p4.tile([128, 2, T], F16, tag="hmsq_sb",
                                   name=f"hmsq{b}"))
        for b in NBR:
            for ec in range(2):
                for t2 in range(2):
                    hp = ps.tile([128, 512], F32, tag="ps",
                                 name=f"hm_ps{b}_{ec}_{t2}")
                    for dc in range(DC):
                        nc.tensor.matmul(
                            hp[:],
                            lhsT=alpha_sb[b][:, dc, ec * 128:(ec + 1) * 128],
                            rhs=bT_sb[b][:, dc, t2 * 512:(t2 + 1) * 512],
                            start=(dc == 0), stop=(dc == DC - 1))
                    sl = slice(t2 * 512, t2 * 512 + 512)
                    nc.vector.tensor_mul(
                        prod_sb[b][:, ec, sl], aT_sb[b][:, ec, sl], hp[:])
                    nc.scalar.activation(hmsq_sb[b][:, ec, sl], hp[:], Square)

        # ---- finals in [t, p] layout (PE, bank ping-pong) + division ----
        for b in NBR:
            fin_ps = [ps.tile([128, (TC // 2) * P], F32, tag="ps",
                              name=f"fin_ps{b}_{i}") for i in range(4)]
            for q, (src, rhs) in enumerate(
                    ((prod_sb[b], w2sal[b]), (hmsq_sb[b], w2sal2[b]))):
                for c in range(TC):
                    fp = fin_ps[q * 2 + c % 2]
                    for dc in range(DC):
                        nc.tensor.matmul(
                            fp[:, (c // 2) * P:(c // 2) * P + P],
                            lhsT=src[:, dc, c * 128:(c + 1) * 128],
                            rhs=rhs[:, dc, :],
                            start=(dc == 0), stop=(dc == DC - 1))
            # persp = num / sqrt((sa+eps)(sh+eps)); h-major (c = 2j+h)
            ssh = pscr.tile([128, 2, 4 * P], F32, tag="ssh", name=f"ssh{b}")
            pnum = pscr.tile([128, 2, 4 * P], F32, tag="pnum", name=f"pnum{b}")
            den = pscr.tile([128, 2, 4 * P], F32, tag="den", name=f"den{b}")
            for h in range(2):
                nc.vector.tensor_copy(ssh[:, h, :], fin_ps[2 + h][:])
                nc.scalar.copy(pnum[:, h, :], fin_ps[h][:])
                nc.vector.tensor_mul(
                    den[:, h, :].rearrange("q (j w) -> q j w", w=P),
                    sa_sb[b][:, h, :].rearrange("q (j w) -> q j w", w=PA)[:, :, 0:P],
                    ssh[:, h, :].rearrange("q (j w) -> q j w", w=P))
            nc.scalar.activation(
                den[:].rearrange("q h jw -> q (h jw)"),
                den[:].rearrange("q h jw -> q (h jw)"), Sqrt, bias=eps_sb[:])
            nc.vector.reciprocal(
                den[:].rearrange("q h jw -> q (h jw)"),
                den[:].rearrange("q h jw -> q (h jw)"))
            persp = pscr.tile([128, 2, 4 * P], F32, tag="persp", name=f"persp{b}")
            for h in range(2):
                nc.vector.tensor_mul(persp[:, h, :], pnum[:, h, :], den[:, h, :])
                nc.sync.dma_start(
                    out=out_d.ap()[b].rearrange(
                        "(q j h) w -> q h j w", h=2, j=4)[:, h],
                    in_=persp[:, h, :].rearrange("q (j w) -> q j w", w=P))

    nc.compile()
    return nc


_NC_CACHE = None


def _get_nc():
    global _NC_CACHE
    if _NC_CACHE is None:
        _NC_CACHE = build_kernel()
    return _NC_CACHE


def make_in_maps(inp_a, inp_b, W):
    w2t = np.ones((D, PA), dtype=np.float32)
    w2t[:, :P] = (W * W).T
    return [
        {"a": inp_a[k * NB:(k + 1) * NB], "b": inp_b[k * NB:(k + 1) * NB],
         "w2t": w2t}
        for k in range(N_CORES)
    ]


def gather_output(res):
    persp = np.concatenate(
        [res.results[k]["out"] for k in range(N_CORES)], axis=0)
    return (persp, persp)


def kernel(inp_a, inp_b, W):
    from concourse.bass_utils import run_bass_kernel_spmd
    inp_a = np.ascontiguousarray(np.asarray(inp_a, dtype=np.float32))
    inp_b = np.ascontiguousarray(np.asarray(inp_b, dtype=np.float32))
    W = np.asarray(W, dtype=np.float32)
    nc = _get_nc()
    in_maps = make_in_maps(inp_a, inp_b, W)
    res = run_bass_kernel_spmd(nc, in_maps, list(range(N_CORES)))
    return gather_output(res)


if __name__ == "__main__":
    rng = np.random.default_rng(0)
    inputs = {
        "inp_a": rng.standard_normal((B, T, D), dtype=np.float32),
        "inp_b": rng.standard_normal((B, T, D), dtype=np.float32),
        "W": rng.uniform(-0.05, 0.05, (P, D)).astype(np.float32),
    }
    out = kernel(**inputs)
    print("ok", out[0].shape, out[0].dtype)



# revision 35
# speedup vs baseline: 1.2088x; 1.2088x over previous
"""AttentiveMatchingLayer TRN2 kernel (v4: wavefront pipeline).

Math (per batch, validated against the jax reference):
  ssa[t] = sum_d a[t,d]^2 ; ssb likewise ; stok = 1/sqrt(ssa*ssb)
  as = a * stok[:,None]                     # carries BOTH l2 norms
  alpha[d,e] = sum_t b[t,d] * as[t,e]       # == ref alpha (norms folded)
  s_al[e] = 1/sqrt(sum_d alpha[d,e]^2)
  hmT[e,t] = sum_d alpha[d,e] * b[t,d]      # s_al folded into w2 scalings
     (differs from ref hmean by a per-token positive factor 1/rb[t],
      which cancels in the final cosine)
  num[t,p] = sum_d (a*hmT) (W2*s_al) ; sa = sum_d a^2 W2 ; sh = sum_d hmT^2 (W2*s_al^2)
  persp = num / sqrt((sa+eps)*(sh+eps))
Sharding: data-parallel over batch B=32 across 8 cores (4 batches/core).

v4 implementation notes:
- Token reindexing t = q*8 + c (q = partition, c = column group) end-to-end:
  loads, compute, and the store share one layout; the store is one DMA per
  batch with 640B-contiguous lines per partition.
- Loads arrive PAIRED (2 batches per SWDGE DMA), issued before everything.
- bT via DMA xbar transpose (1 instr/batch, SBUF->SBUF f16, layout
  [d_p, j=c*2+dc, q]); aT via PE transposes into a transient PSUM tile,
  immediately evacuated (DVE copy -> aT_sb, ACT Square -> asq_sb).
- ssb via bsq (Pool squares of bT_j, quarter-split for latency) + ones-
  matmuls on PE; ssa rides the augmented ones column of w2t in the sa
  matmul. sa/ssb/sal/num/sh all pack into ONE PSUM bank per batch.
- Wavefront emission S1/S2/S3 staggered two batches deep so every engine
  always has ready work; PSUM: tr(2) + hm(2) + al(2) + smalls(2) = 8 banks.
"""

import numpy as np
from contextlib import ExitStack

import concourse.bacc as bacc
import concourse.bass as bass
import concourse.tile as tile
from concourse import masks, mybir

B, T, D, P = 32, 1024, 256, 20
PA = P + 1          # w2t augmented with a ones column (-> ssa)
PW = PA + 1         # smalls row group: 21 sa cols + 1 ssb col
SM_SAL = 8 * PW     # 176: sal offset in smalls
SM_NUM = SM_SAL + 2  # 178: num region [178:338]
SM_SH = SM_NUM + 160  # 338: sh region [338:498]
SM_W = SM_SH + 160   # 498 f32 = 1992B, fits one 2KB PSUM bank
N_CORES = 8
NB = B // N_CORES
TC = T // 128       # 8 token column-groups (c)
DC = D // 128       # 2 d chunks
F32 = mybir.dt.float32
F16 = mybir.dt.float16
EPS = 1e-12
Square = mybir.ActivationFunctionType.Square
Sqrt = mybir.ActivationFunctionType.Sqrt
from concourse.alu_op_type import AluOpType as _Alu
Adiv = _Alu.divide


def build_kernel():
    nc = bacc.Bacc("TRN2", target_bir_lowering=False, debug=False,
                   num_devices=N_CORES, dynamic_dma_scratch_size=32768)
    a_in = nc.declare_dram_parameter("a", [NB, T, D], F32, isOutput=False)
    b_in = nc.declare_dram_parameter("b", [NB, T, D], F32, isOutput=False)
    w2t_in = nc.declare_dram_parameter("w2t", [D, PA], F32, isOutput=False)
    out_d = nc.declare_dram_parameter("out", [NB, T, P], F32, isOutput=True)

    with tile.TileContext(nc) as tc, ExitStack() as ctx:
        consts = ctx.enter_context(tc.tile_pool(name="consts", bufs=1))
        pp = ctx.enter_context(tc.tile_pool(name="pp", bufs=2))
        p4 = ctx.enter_context(tc.tile_pool(name="p4", bufs=NB))
        # PSUM: tr(2) + al(2) + hm(2) + smalls(2) = 8 banks
        pstr = ctx.enter_context(tc.tile_pool(name="pstr", bufs=1,
                                              space="PSUM"))
        ps2 = ctx.enter_context(tc.tile_pool(name="ps2", bufs=2,
                                             space="PSUM"))

        identf = consts.tile([128, 128], F32)
        masks.make_identity(nc, identf[:])
        ident = consts.tile([128, 128], F16)
        nc.vector.tensor_copy(ident[:], identf[:])
        ones = consts.tile([128, 1], F16)
        nc.vector.memset(ones[:], 1.0)
        eps_sb = consts.tile([128, 1], F32)
        nc.vector.memset(eps_sb[:], EPS)
        w2t = consts.tile([128, DC, PA], F16)
        nc.gpsimd.dma_start(
            out=w2t[:], in_=w2t_in.ap().rearrange("(dc p) w -> p dc w", p=128))

        NBR = range(NB)
        J = TC * DC

        # ---- paired loads (b then a per pair); pair 0 up front, pair 1
        # emitted inside S1(1) so batch-0's Pool work dispatches first ----
        a_pair, b_pair = [None] * (NB // 2), [None] * (NB // 2)

        def load_pair(pr):
            bp = pp.tile([128, 2, TC, D], F16, tag="b_pair", name=f"bp{pr}")
            nc.gpsimd.dma_start(
                out=bp[:],
                in_=b_in.ap()[2 * pr:2 * pr + 2].rearrange(
                    "n (q c) d -> q n c d", q=128))
            b_pair[pr] = bp
            ap_ = pp.tile([128, 2, TC, D], F16, tag="a_pair", name=f"ap{pr}")
            nc.gpsimd.dma_start(
                out=ap_[:],
                in_=a_in.ap()[2 * pr:2 * pr + 2].rearrange(
                    "n (q c) d -> q n c d", q=128))
            a_pair[pr] = ap_

        load_pair(0)

        # Per-batch state carried between stages
        st_ = {}

        def S1(b):
            a_sb = a_pair[b // 2][:, b % 2]
            b_sb = b_pair[b // 2][:, b % 2]

            bT_j = p4.tile([128, J, 128], F16, tag="bT_j", name=f"bT{b}")
            nc.sync.dma_start_transpose(out=bT_j[:], in_=b_sb)

            # aT via PE into transient PSUM; evac immediately
            tr = pstr.tile([128, DC, 1024], F16, tag="tr", name=f"tr{b}")
            for dc in range(DC):
                for c in range(TC):
                    nc.tensor.transpose(
                        out=tr[:, dc, c * 128:(c + 1) * 128],
                        in_=a_sb[:, c, dc * 128:(dc + 1) * 128],
                        identity=ident[:])
            aT = p4.tile([128, DC, 1024], F16, tag="aT", name=f"aT{b}")
            nc.vector.tensor_copy(
                aT[:].rearrange("p x q -> p (x q)"),
                tr[:].rearrange("p x q -> p (x q)"))
            asq = p4.tile([128, DC, 1024], F16, tag="asq", name=f"asq{b}")
            nc.scalar.activation(
                asq[:].rearrange("p x q -> p (x q)"),
                tr[:].rearrange("p x q -> p (x q)"), Square)

            # bsq on Pool, two c-halves for latency
            bsq = p4.tile([128, J, 128], F16, tag="bsq", name=f"bsq{b}")
            for ch in range(2):
                sl = slice(ch * 8, ch * 8 + 8)
                nc.gpsimd.tensor_mul(
                    bsq[:, sl, :], bT_j[:, sl, :], bT_j[:, sl, :])

            # smalls bank: sa/ssb groups + sal + num + sh
            sm = ps2.tile([128, SM_W], F32, tag="smalls", name=f"smalls{b}")
            sa_ps = sm[:, 0:8 * PW].rearrange("p (c w) -> p c w", w=PW)
            for c in range(TC):
                for dc in range(DC):
                    nc.tensor.matmul(
                        sa_ps[:, c, PA:PW],
                        lhsT=bsq[:, c * 2 + dc, :],
                        rhs=ones[:],
                        start=(dc == 0), stop=(dc == DC - 1))
            for c in range(TC):
                for dc in range(DC):
                    nc.tensor.matmul(
                        sa_ps[:, c, 0:PA],
                        lhsT=asq[:, dc, c * 128:(c + 1) * 128],
                        rhs=w2t[:, dc, :],
                        start=(dc == 0), stop=(dc == DC - 1))

            # evac sa/ssa/ssb region to SBUF (vector ops may read only
            # one PSUM operand; stok and den each need two of these)
            sa_sb = p4.tile([128, TC, PW], F32, tag="sa_sb", name=f"sasb{b}")
            nc.scalar.activation(
                sa_sb[:].rearrange("p c w -> p (c w)"),
                sm[:, 0:TC * PW],
                mybir.ActivationFunctionType.Copy)
            stok = p4.tile([128, TC], F32, tag="stok", name=f"stok{b}")
            nc.vector.tensor_mul(stok[:], sa_sb[:, :, P], sa_sb[:, :, PA])
            nc.scalar.activation(stok[:], stok[:], Sqrt)
            nc.vector.reciprocal(stok[:], stok[:])
            as_sb = p4.tile([128, TC, D], F16, tag="as_sb", name=f"as_sb{b}")
            for c in range(TC):
                nc.vector.tensor_scalar_mul(
                    as_sb[:, c, :], a_sb[:, c, :], stok[:, c:c + 1])

            st_[b] = dict(a_sb=a_sb, b_sb=b_sb, bT_j=bT_j, aT=aT,
                          sm=sm, sa_sb=sa_sb, as_sb=as_sb)

        def S2(b):
            s = st_[b]
            b_sb, as_sb, sm = s["b_sb"], s["as_sb"], s["sm"]
            al_ps = ps2.tile([128, DC, 256], F32, tag="al_ps",
                             name=f"al_ps{b}")
            for dc in range(DC):
                for c in range(TC):
                    nc.tensor.matmul(
                        al_ps[:, dc, :],
                        lhsT=b_sb[:, c, dc * 128:(dc + 1) * 128],
                        rhs=as_sb[:, c, :],
                        start=(c == 0), stop=(c == TC - 1))
            alpha = p4.tile([128, DC, 256], F16, tag="alpha", name=f"alpha{b}")
            nc.vector.tensor_copy(
                alpha[:].rearrange("p x e -> p (x e)"),
                al_ps[:].rearrange("p x e -> p (x e)"))
            alsq = p4.tile([128, DC, 256], F16, tag="alsq", name=f"alsq{b}")
            nc.scalar.activation(
                alsq[:].rearrange("p x e -> p (x e)"),
                al_ps[:].rearrange("p x e -> p (x e)"), Square)

            sal_ps = sm[:, SM_SAL:SM_SAL + 2]
            for ec in range(2):
                for dc in range(DC):
                    nc.tensor.matmul(
                        sal_ps[:, ec:ec + 1],
                        lhsT=alsq[:, dc, ec * 128:(ec + 1) * 128],
                        rhs=ones[:],
                        start=(dc == 0), stop=(dc == DC - 1))
            sal = p4.tile([128, 2], F32, tag="sal", name=f"sal{b}")
            nc.scalar.activation(sal[:], sal_ps[:], Sqrt)
            nc.vector.reciprocal(sal[:], sal[:])
            ws = p4.tile([128, DC, P], F16, tag="ws", name=f"ws{b}")
            ws2 = p4.tile([128, DC, P], F16, tag="ws2", name=f"ws2{b}")
            for ec in range(DC):
                nc.vector.tensor_scalar_mul(
                    ws[:, ec, :], w2t[:, ec, 0:P], sal[:, ec:ec + 1])
                nc.vector.tensor_scalar_mul(
                    ws2[:, ec, :], ws[:, ec, :], sal[:, ec:ec + 1])
            s.update(alpha=alpha, ws=ws, ws2=ws2)

        def S3(b):
            s = st_[b]
            bT_j, aT, alpha, sm = s["bT_j"], s["aT"], s["alpha"], s["sm"]
            sa_sb, ws, ws2 = s["sa_sb"], s["ws"], s["ws2"]
            prod = p4.tile([128, 2, 1024], F16, tag="prod", name=f"prod{b}")
            hmsq = p4.tile([128, 2, 1024], F16, tag="hmsq", name=f"hmsq{b}")
            for ec in range(2):
                for th in range(2):
                    hm_ps = ps2.tile([128, 512], F32, tag="hm_ps",
                                     name=f"hm{b}_{ec}{th}")
                    for dc in range(DC):
                        nc.tensor.matmul(
                            hm_ps[:],
                            lhsT=alpha[:, dc, ec * 128:(ec + 1) * 128],
                            rhs=bT_j[:, th * 8 + dc:th * 8 + 8:2, :],
                            start=(dc == 0), stop=(dc == DC - 1))
                    sl = slice(th * 512, th * 512 + 512)
                    nc.vector.tensor_mul(
                        prod[:, ec, sl], aT[:, ec, sl], hm_ps[:])
                    nc.scalar.activation(hmsq[:, ec, sl], hm_ps[:], Square)

            num_ps = sm[:, SM_NUM:SM_NUM + 160].rearrange(
                "p (c w) -> p c w", w=P)
            sh_ps = sm[:, SM_SH:SM_SH + 160].rearrange(
                "p (c w) -> p c w", w=P)
            for c in range(TC):
                for ec in range(2):
                    nc.tensor.matmul(
                        num_ps[:, c, :],
                        lhsT=prod[:, ec, c * 128:(c + 1) * 128],
                        rhs=ws[:, ec, :],
                        start=(ec == 0), stop=(ec == 1))
                for ec in range(2):
                    nc.tensor.matmul(
                        sh_ps[:, c, :],
                        lhsT=hmsq[:, ec, c * 128:(c + 1) * 128],
                        rhs=ws2[:, ec, :],
                        start=(ec == 0), stop=(ec == 1))
            den = p4.tile([128, TC, P], F32, tag="den", name=f"den{b}")
            nc.vector.tensor_mul(den[:], sa_sb[:, :, 0:P], sh_ps)
            nc.scalar.activation(
                den[:].rearrange("p c w -> p (c w)"),
                den[:].rearrange("p c w -> p (c w)"), Sqrt, bias=eps_sb[:])
            nc.vector.reciprocal(
                den[:].rearrange("p c w -> p (c w)"),
                den[:].rearrange("p c w -> p (c w)"))
            persp = p4.tile([128, TC, P], F32, tag="persp", name=f"persp{b}")
            nc.vector.tensor_mul(persp[:], num_ps, den[:])
            s["persp"] = persp

        # Wavefront: two-batch stagger matching the bufs=2 PSUM tags.
        S1(0)
        load_pair(1)
        S1(1); S2(0); S3(0)
        S1(2); S2(1); S3(1)
        S1(3); S2(2); S3(2)
        S2(3); S3(3)

        for b in NBR:
            nc.sync.dma_start(
                out=out_d.ap()[b].rearrange("(q c) w -> q c w", q=128),
                in_=st_[b]["persp"][:])

    nc.compile()
    return nc


_NC_CACHE = None


def _get_nc():
    global _NC_CACHE
    if _NC_CACHE is None:
        _NC_CACHE = build_kernel()
    return _NC_CACHE


def make_in_maps(inp_a, inp_b, W):
    w2t = np.ones((D, PA), dtype=np.float32)
    w2t[:, :P] = (W * W).T
    return [
        {"a": inp_a[k * NB:(k + 1) * NB], "b": inp_b[k * NB:(k + 1) * NB],
         "w2t": w2t}
        for k in range(N_CORES)
    ]


def gather_output(res):
    persp = np.concatenate(
        [res.results[k]["out"] for k in range(N_CORES)], axis=0)
    return (persp, persp)


def kernel(inp_a, inp_b, W):
    from concourse.bass_utils import run_bass_kernel_spmd
    inp_a = np.ascontiguousarray(np.asarray(inp_a, dtype=np.float32))
    inp_b = np.ascontiguousarray(np.asarray(inp_b, dtype=np.float32))
    W = np.asarray(W, dtype=np.float32)
    nc = _get_nc()
    in_maps = make_in_maps(inp_a, inp_b, W)
    res = run_bass_kernel_spmd(nc, in_maps, list(range(N_CORES)))
    return gather_output(res)


if __name__ == "__main__":
    rng = np.random.default_rng(0)
    inputs = {
        "inp_a": rng.standard_normal((B, T, D), dtype=np.float32),
        "inp_b": rng.standard_normal((B, T, D), dtype=np.float32),
        "W": rng.uniform(-0.05, 0.05, (P, D)).astype(np.float32),
    }
    out = kernel(**inputs)
    print("ok", out[0].shape, out[0].dtype)
